# revision 25
# baseline (speedup 1.0000x reference)
"""Trainium2 Bass kernel for nn_NoiseGenerator.

Math (verified against the jax reference on host):
  The reference's irfft -> fftshift -> hann-window -> slice pipeline is a fixed
  linear map of the 8192 spectral magnitudes into a 1023-tap impulse:
      impulse = noise_bands @ C,
      C[k, j] = w_j * alpha_k * cos(2*pi*k*(7681 + j) / 16382)
      w_j = 0.5 - 0.5*cos(2*pi*(j+2)/1024),  alpha = [1, 2, ..., 2, 1] / 16382
  The fft_convolve + crop is then an ordinary linear convolution of noise row 0
  with each batch row's impulse:
      out[b, t] = amps_b / M * sum_i f[b, i] * n0u[t - 512 + i]
      f[b, i] = impulse[b, 1022 - i] (reversal folded into C), n0u = mean_0 + eps[0]
      M = max(mean + eps)  (global max over the full batch)

Distribution over 8 cores: data-parallel over batch (128 rows/core), two
phases with host-mediated exchange (no device collectives -- measured ~100us
of barrier/CC overhead for tiny payloads on this fabric):
  phase 1 (per core): MLP mean head, shard-local max of mean+eps, and a
    128-column slice of W_ic = [W_coeff; b_coeff] @ C_rev (C column-sharded,
    fp16 with a 2^13 scale, unscaled on-device after the accumulation).
  host: concatenates the 8 W_ic slices (pure gather), takes max of the 8
    shard maxima (8 floats), picks mean[0] from core 0's output.
  phase 2 (per core): amps head, filter rows fT = W_ic^T x_scaled^T with
    amps/M (and a 2^8 fp16-headroom scale) folded in, Toeplitz-matmul
    convolution against fp16 shifted windows of noise row 0, 2^-8 applied on
    the PSUM drain, streamed to the output.
"""

import os

import numpy as np

B = 1024
D = 64
DA = D + 2  # vel feats + K/MAX_STEPS + const-1 (folds the coeff bias)
W = 8192
NFFT = 16382
IRP = 1024  # impulse taps padded 1023 -> 1024
NCORES = 8
BSH = B // NCORES
MAX_STEPS = 2799.0
TPAD = 512  # left zero-padding of n0
N0PAD = 9216  # 512 + 8192 + 512 zeros
TW = 9088  # Toeplitz tile free size: 8192 + 7*128 + 512 - 512
CSCALE = 8192.0  # fp16 range lift for C (|C| <= 1.3e-4)
FSCALE = 256.0  # fp16 range lift for the conv filter rows

_CACHE = {}


def _build_crev() -> np.ndarray:
    """C_rev [8192, 1024] f32: column i equals C[:, 1022-i]; column 1023 is 0."""
    k = np.arange(W, dtype=np.float64)
    alpha = np.full(W, 2.0)
    alpha[0] = 1.0
    alpha[-1] = 1.0
    alpha /= NFFT
    j = np.arange(1023, dtype=np.float64)
    wj = 0.5 - 0.5 * np.cos(2.0 * np.pi * (j + 2.0) / 1024.0)
    ang = (2.0 * np.pi / NFFT) * np.outer(k, 7681.0 + j)
    C = (alpha[:, None] * np.cos(ang)) * wj[None, :]
    crev = np.zeros((W, IRP), dtype=np.float64)
    crev[:, :1023] = C[:, ::-1]
    return np.ascontiguousarray(crev, dtype=np.float32)


def _begin_program():
    import concourse.mybir as mybir
    import concourse.tile as tile
    from concourse import bacc

    nc = bacc.Bacc("TRN2", target_bir_lowering=False, debug=False, num_devices=NCORES)
    return nc, tile, mybir


def _build_x_aug(nc, const, vel, kk, f32):
    """x_aug [128, DA] = [vel, K/MAX_STEPS, 1]."""
    x_aug = const.tile([128, DA], f32)
    nc.sync.dma_start(x_aug[:, 0:D], vel)
    nc.sync.dma_start(x_aug[:, D : D + 1], kk)
    nc.scalar.mul(x_aug[:, D : D + 1], x_aug[:, D : D + 1], 1.0 / MAX_STEPS)
    nc.vector.memset(x_aug[:, D + 1 : DA], 1.0)
    return x_aug


def _build_prog1():
    """Per-core: mean head, local max of mean+eps, W_ic column slice."""
    from contextlib import ExitStack

    nc, tile, mybir = _begin_program()
    f32 = mybir.dt.float32
    f16 = mybir.dt.float16
    AFT = mybir.ActivationFunctionType
    X = mybir.AxisListType.X

    vel = nc.dram_tensor("vel", [BSH, D], f32, kind="ExternalInput").ap()
    kk = nc.dram_tensor("kk", [BSH, 1], f32, kind="ExternalInput").ap()
    eps_sh = nc.dram_tensor("eps_sh", [BSH, W], f32, kind="ExternalInput").ap()
    # host-prelaid SBUF layouts: wsb[p, DA*c + d] = W_aug^T[128c + p, d] (fp16)
    wsb_d = nc.dram_tensor("wsb", [128, 64 * DA], f16, kind="ExternalInput").ap()
    # csb[p, 128c + i] = CSCALE * C_rev[128c + p, my_slice + i] (fp16)
    csb_d = nc.dram_tensor("csb", [128, 64 * BSH], f16, kind="ExternalInput").ap()
    w_mean = nc.dram_tensor("w_mean", [DA, 1], f32, kind="ExternalInput").ap()
    w_amps = nc.dram_tensor("w_amps", [DA, 1], f32, kind="ExternalInput").ap()
    ident_in = nc.dram_tensor("ident", [128, 128], f32, kind="ExternalInput").ap()
    mean_out = nc.dram_tensor("mean_out", [BSH, 1], f32, kind="ExternalOutput").ap()
    amps_out = nc.dram_tensor("amps_out", [BSH, 1], f32, kind="ExternalOutput").ap()
    lmax_out = nc.dram_tensor("lmax_out", [1, 1], f32, kind="ExternalOutput").ap()
    nmean0_out = nc.dram_tensor("nmean0_out", [1, 1], f32, kind="ExternalOutput").ap()
    wic_out = nc.dram_tensor("wic_out", [DA, BSH], f32, kind="ExternalOutput").ap()

    NG = 8  # csb DMA groups for DMA/matmul overlap

    with tile.TileContext(nc) as tc, ExitStack() as ctx:
        const = ctx.enter_context(tc.tile_pool(name="const", bufs=1))
        work = ctx.enter_context(tc.tile_pool(name="work", bufs=4))
        pmisc = ctx.enter_context(tc.tile_pool(name="pmisc", bufs=2, space="PSUM"))
        pwic = ctx.enter_context(tc.tile_pool(name="pwic", bufs=1, space="PSUM"))

        # small critical loads first on the sync ring
        x_aug = _build_x_aug(nc, const, vel, kk, f32)
        wme = const.tile([DA, 1], f32)
        nc.sync.dma_start(wme[:], w_mean)
        wam = const.tile([DA, 1], f32)
        nc.sync.dma_start(wam[:], w_amps)

        # big streams on the scalar (ACT) HWDGE ring, ident first (tiny)
        ident = const.tile([128, 128], f32)
        nc.scalar.dma_start(ident[:], ident_in)
        wsb = const.tile([128, 64 * DA], f16)
        nc.scalar.dma_start(wsb[:], wsb_d)
        csb = const.tile([128, 64 * BSH], f16)
        GW = 64 * BSH // NG
        for g in range(NG):
            nc.scalar.dma_start(
                csb[:, GW * g : GW * (g + 1)], csb_d[:, GW * g : GW * (g + 1)]
            )

        xT_ps = pmisc.tile([DA, 128], f32, tag="misc")
        nc.tensor.transpose(xT_ps[:], x_aug[:], ident[:])
        xT = const.tile([DA, 128], f32)
        nc.vector.tensor_copy(xT[:], xT_ps[:])

        mean_ps = pmisc.tile([128, 1], f32, tag="misc")
        nc.tensor.matmul(mean_ps[:], lhsT=xT[:], rhs=wme[:], start=True, stop=True)
        mean_sb = const.tile([128, 1], f32)
        nc.scalar.activation(mean_sb[:], mean_ps[:], AFT.Tanh)
        nc.scalar.dma_start(mean_out, mean_sb[:])
        nmean_sb = const.tile([1, 1], f32)
        nc.scalar.mul(nmean_sb[:], mean_sb[0:1, 0:1], -1.0)
        nc.scalar.dma_start(nmean0_out, nmean_sb[:])

        amps_ps = pmisc.tile([128, 1], f32, tag="misc")
        nc.tensor.matmul(amps_ps[:], lhsT=xT[:], rhs=wam[:], start=True, stop=True)
        amps_sb = const.tile([128, 1], f32)
        nc.scalar.activation(amps_sb[:], amps_ps[:], AFT.Sigmoid)
        nc.scalar.dma_start(amps_out, amps_sb[:])

        # local max of (mean_b + eps_b[w]) over this shard
        rm4 = const.tile([128, 4], f32)
        for i in range(4):
            ch = work.tile([128, 2048], f32, tag="epschunk")
            nc.sync.dma_start(ch[:], eps_sh[:, 2048 * i : 2048 * (i + 1)])
            nc.vector.reduce_max(rm4[:, i : i + 1], ch[:], axis=X)
        rm1 = const.tile([128, 1], f32)
        nc.vector.reduce_max(rm1[:], rm4[:], axis=X)
        nc.vector.tensor_add(rm1[:], rm1[:], mean_sb[:])
        rmT_ps = pmisc.tile([1, 128], f32, tag="misc")
        nc.tensor.transpose(rmT_ps[:], rm1[:], ident[:])
        lmax_sb = const.tile([1, 1], f32)
        nc.vector.reduce_max(lmax_sb[:], rmT_ps[:], axis=X)
        nc.scalar.dma_start(lmax_out, lmax_sb[0:1, 0:1])

        # W_ic slice: [W_coeff; b_coeff] @ C_rev[:, my 128 cols], grouped to
        # overlap with the csb stream
        wic_ps = pwic.tile([DA, 128], f32, tag="wic")
        for c in range(64):
            nc.tensor.matmul(
                wic_ps[:],
                lhsT=wsb[:, DA * c : DA * (c + 1)],
                rhs=csb[:, BSH * c : BSH * (c + 1)],
                start=(c == 0),
                stop=(c == 63),
            )
        wic_sb = work.tile([DA, BSH], f32)
        nc.vector.tensor_scalar_mul(wic_sb[:], wic_ps[:], 1.0 / CSCALE)
        nc.scalar.dma_start(wic_out, wic_sb[:])

    nc.compile()
    return nc


def _build_prog2():
    """Per-core: scaled filter rows, Toeplitz-matmul convolution.

    The Toeplitz tile streams straight from the host-padded fp16 noise row
    (padding holds -mean0, so out-of-range taps cancel); the +mean0 term is
    folded in exactly as mean0 * F[b] (F = filter-row sums) on the PSUM drain.
    """
    from contextlib import ExitStack

    import concourse.bass as bass

    nc, tile, mybir = _begin_program()
    f32 = mybir.dt.float32
    f32r = mybir.dt.float32r
    f16 = mybir.dt.float16

    vel = nc.dram_tensor("vel", [BSH, D], f32, kind="ExternalInput").ap()
    kk = nc.dram_tensor("kk", [BSH, 1], f32, kind="ExternalInput").ap()
    eps0pad = nc.dram_tensor("eps0pad", [1, N0PAD], f16, kind="ExternalInput").ap()
    amps_in = nc.dram_tensor("amps_in", [BSH, 1], f32, kind="ExternalInput").ap()
    mean0_in = nc.dram_tensor("mean0_in", [1, 1], f32, kind="ExternalInput").ap()
    m_in = nc.dram_tensor("m_in", [1, 1], f32, kind="ExternalInput").ap()
    wic_full = nc.dram_tensor("wic_full", [DA, IRP], f32, kind="ExternalInput").ap()
    ident_in = nc.dram_tensor("ident", [128, 128], f32, kind="ExternalInput").ap()
    out_noise = nc.dram_tensor("out_noise", [BSH, W], f32, kind="ExternalOutput").ap()

    with tile.TileContext(nc) as tc, ExitStack() as ctx:
        const = ctx.enter_context(tc.tile_pool(name="const", bufs=1))
        work = ctx.enter_context(tc.tile_pool(name="work", bufs=3))
        pmisc = ctx.enter_context(tc.tile_pool(name="pmisc", bufs=2, space="PSUM"))
        pconv = ctx.enter_context(tc.tile_pool(name="pconv", bufs=4, space="PSUM"))
        pft = ctx.enter_context(tc.tile_pool(name="pft", bufs=2, space="PSUM"))

        # ---- Toeplitz pieces stream immediately from the padded input ----
        t_sb = const.tile([128, TW], f16)
        NP = 4
        PW = TW // NP
        for k in range(NP):
            lo = PW * k
            hi = PW * (k + 1) if k < NP - 1 else TW
            piece = bass.AP(eps0pad.tensor, lo, [[1, 128], [1, hi - lo]])
            eng = nc.sync if k % 2 == 0 else nc.scalar
            eng.dma_start(t_sb[:, lo:hi], piece)

        # small loads
        m0_sb = const.tile([1, 1], f32)
        nc.sync.dma_start(m0_sb[:], mean0_in)
        amps_sb = const.tile([128, 1], f32)
        nc.sync.dma_start(amps_sb[:], amps_in)
        gmax = const.tile([1, 1], f32)
        nc.sync.dma_start(gmax[:], m_in)
        x_aug = _build_x_aug(nc, const, vel, kk, f32)
        wic_sb = const.tile([DA, IRP], f32r)
        nc.scalar.dma_start(wic_sb[:], wic_full.bitcast(f32r))
        ident = const.tile([128, 128], f32)
        nc.scalar.dma_start(ident[:], ident_in)

        ones_row = const.tile([1, 128], f32)
        nc.vector.memset(ones_row[:], 1.0)
        m0bc_ps = pmisc.tile([128, 1], f32, tag="misc")
        nc.tensor.matmul(m0bc_ps[:], lhsT=ones_row[:], rhs=m0_sb[:], start=True, stop=True)
        m0bc = const.tile([128, 1], f32)
        nc.vector.tensor_copy(m0bc[:], m0bc_ps[:])

        # s_b = FSCALE * amps_b / M  (M broadcast via ones-matmul)
        gmax_ps = pmisc.tile([128, 1], f32, tag="misc")
        nc.tensor.matmul(gmax_ps[:], lhsT=ones_row[:], rhs=gmax[:], start=True, stop=True)
        minv = const.tile([128, 1], f32)
        nc.vector.reciprocal(minv[:], gmax_ps[:])
        s_sb = const.tile([128, 1], f32)
        nc.vector.tensor_mul(s_sb[:], amps_sb[:], minv[:])
        nc.scalar.mul(s_sb[:], s_sb[:], FSCALE)

        xs = const.tile([128, DA], f32)
        nc.vector.tensor_scalar_mul(xs[:], x_aug[:], s_sb[:])
        xsT_ps = pmisc.tile([DA, 128], f32, tag="misc")
        nc.tensor.transpose(xsT_ps[:], xs[:], ident[:])
        xsT = const.tile([DA, 128], f32r)
        nc.vector.tensor_copy(xsT[:], xsT_ps[:])

        # filter rows fT[i, b] = sum_d W_ic[d, i] * xs[b, d]  (fp16, x FSCALE)
        fT = const.tile([128, IRP], f16)
        for c in range(8):
            fp = pft.tile([128, 128], f32, tag="fp")
            nc.tensor.matmul(
                fp[:],
                lhsT=wic_sb[:, 128 * c : 128 * (c + 1)],
                rhs=xsT[:],
                start=True,
                stop=True,
            )
            nc.vector.tensor_copy(fT[:, 128 * c : 128 * (c + 1)], fp[:])

        # F[b] = sum_i fT[i, b]; drain adds mean0 * F before unscaling
        ones16 = const.tile([128, 1], f16)
        nc.vector.memset(ones16[:], 1.0)
        f_ps = pmisc.tile([128, 1], f32, tag="misc")
        for c in range(8):
            nc.tensor.matmul(
                f_ps[:],
                lhsT=fT[:, 128 * c : 128 * (c + 1)],
                rhs=ones16[:],
                start=(c == 0),
                stop=(c == 7),
            )
        mF = const.tile([128, 1], f32)
        nc.vector.tensor_mul(mF[:], f_ps[:], m0bc[:])

        # ---- conv: out[b,t] = (sum_i fT[i,b] * T[.] + FSCALE*m0*F[b]) / FSCALE
        for t in range(16):
            po = pconv.tile([128, 512], f32, tag="conv")
            for c in range(8):
                nc.tensor.matmul(
                    po[:],
                    lhsT=fT[:, 128 * c : 128 * (c + 1)],
                    rhs=t_sb[:, 128 * c + 512 * t : 128 * c + 512 * t + 512],
                    start=(c == 0),
                    stop=(c == 7),
                )
            ob = work.tile([128, 512], f32, tag="outbounce")
            nc.vector.tensor_scalar(
                ob[:],
                po[:],
                mF[:],
                1.0 / FSCALE,
                op0=mybir.AluOpType.add,
                op1=mybir.AluOpType.mult,
            )
            nc.scalar.dma_start(out_noise[:, 512 * t : 512 * (t + 1)], ob[:])

    nc.compile()
    return nc


def _get_progs():
    if "nc1" not in _CACHE:
        _CACHE["nc1"] = _build_prog1()
        _CACHE["nc2"] = _build_prog2()
    return _CACHE["nc1"], _CACHE["nc2"]


def _get_crev16():
    """(CSCALE * C_rev) as fp16, chunk-relaid: [8192, 1024] -> [64, 128, 1024]."""
    if "crev16" not in _CACHE:
        crev = _build_crev()
        _CACHE["crev16"] = np.ascontiguousarray(
            (crev * CSCALE).astype(np.float16).reshape(64, 128, IRP)
        )
    return _CACHE["crev16"]


def _prep(inputs: dict) -> dict:
    p = {}
    if "ident" not in _CACHE:
        _CACHE["ident"] = np.ascontiguousarray(np.eye(128, dtype=np.float32))
    p["ident"] = _CACHE["ident"]
    p["vel"] = np.ascontiguousarray(np.asarray(inputs["vel_inputs"]), dtype=np.float32)
    p["K"] = np.ascontiguousarray(np.asarray(inputs["K"]), dtype=np.float32)
    p["eps"] = np.ascontiguousarray(np.asarray(inputs["eps"]), dtype=np.float32)
    w_coeff = np.asarray(inputs["W_coeff"], dtype=np.float32)
    b_coeff = np.asarray(inputs["b_coeff"], dtype=np.float32)
    w_aug_t = np.concatenate([w_coeff.T, b_coeff[:, None]], axis=1)  # [W, DA]
    # wsb[p, DA*c + d] = w_aug_t[128c + p, d], fp16
    p["wsb"] = np.ascontiguousarray(
        w_aug_t.reshape(64, 128, DA).transpose(1, 0, 2).reshape(128, 64 * DA)
    ).astype(np.float16)
    p["w_amps"] = np.ascontiguousarray(
        np.concatenate(
            [np.asarray(inputs["W_amps"], np.float32), np.asarray(inputs["b_amps"], np.float32)[:, None]],
            axis=0,
        )
    )
    p["w_mean"] = np.ascontiguousarray(
        np.concatenate(
            [np.asarray(inputs["W_mean"], np.float32), np.asarray(inputs["b_mean"], np.float32)[:, None]],
            axis=0,
        )
    )
    return p


def make_in_maps1(p: dict) -> list:
    crev16 = _get_crev16()  # [64, 128, IRP]
    maps = []
    for c in range(NCORES):
        csb = np.ascontiguousarray(
            crev16[:, :, BSH * c : BSH * (c + 1)]
            .transpose(1, 0, 2)
            .reshape(128, 64 * BSH)
        )
        sl = slice(BSH * c, BSH * (c + 1))
        maps.append(
            {
                "vel": np.ascontiguousarray(p["vel"][sl]),
                "kk": np.ascontiguousarray(p["K"][sl]),
                "eps_sh": np.ascontiguousarray(p["eps"][sl]),
                "wsb": p["wsb"],
                "csb": csb,
                "w_mean": p["w_mean"],
                "w_amps": p["w_amps"],
                "ident": p["ident"],
            }
        )
    return maps


def glue12(results1: list) -> tuple:
    """Host-side exchange: gather W_ic slices, max of shard maxima, mean row 0."""
    wic_full = np.ascontiguousarray(
        np.concatenate([r["wic_out"] for r in results1], axis=1)
    )  # [DA, 1024]
    m = np.max([r["lmax_out"][0, 0] for r in results1]).reshape(1, 1).astype(np.float32)
    mean = np.concatenate([r["mean_out"] for r in results1], axis=0)  # [B, 1]
    mean0 = np.ascontiguousarray(mean[0:1, 0:1])
    amps = [np.ascontiguousarray(r["amps_out"]) for r in results1]
    nmean0 = results1[0]["nmean0_out"][0, 0]
    return wic_full, m, mean, mean0, amps, nmean0


def make_in_maps2(
    p: dict,
    wic_full: np.ndarray,
    m: np.ndarray,
    mean0: np.ndarray,
    amps: list,
    nmean0,
) -> list:
    eps0pad = np.full((1, N0PAD), nmean0, dtype=np.float16)
    eps0pad[0, TPAD : TPAD + W] = p["eps"][0].astype(np.float16)
    eps0pad = np.ascontiguousarray(eps0pad)
    maps = []
    for c in range(NCORES):
        sl = slice(BSH * c, BSH * (c + 1))
        maps.append(
            {
                "vel": np.ascontiguousarray(p["vel"][sl]),
                "kk": np.ascontiguousarray(p["K"][sl]),
                "eps0pad": eps0pad,
                "amps_in": amps[c],
                "mean0_in": mean0,
                "m_in": m,
                "wic_full": wic_full,
                "ident": p["ident"],
            }
        )
    return maps


def kernel(**inputs):
    from concourse.bass_utils import run_bass_kernel_spmd

    nc1, nc2 = _get_progs()
    p = _prep(inputs)
    trace = os.environ.get("NOISE_KERNEL_TRACE", "0") == "1"
    core_ids = list(range(NCORES))

    res1 = run_bass_kernel_spmd(nc1, make_in_maps1(p), core_ids=core_ids, trace=trace)
    wic_full, m, mean, mean0, amps, nmean0 = glue12(res1.results)
    res2 = run_bass_kernel_spmd(
        nc2,
        make_in_maps2(p, wic_full, m, mean0, amps, nmean0),
        core_ids=core_ids,
        trace=trace,
    )
    _CACHE["last_result1"] = res1
    _CACHE["last_result2"] = res2
    out = np.concatenate([r["out_noise"] for r in res2.results], axis=0)
    return out, mean


# revision 27
# speedup vs baseline: 1.0086x; 1.0086x over previous
"""Trainium2 Bass kernel for nn_NoiseGenerator.

Math (verified against the jax reference on host):
  The reference's irfft -> fftshift -> hann-window -> slice pipeline is a fixed
  linear map of the 8192 spectral magnitudes into a 1023-tap impulse:
      impulse = noise_bands @ C,
      C[k, j] = w_j * alpha_k * cos(2*pi*k*(7681 + j) / 16382)
      w_j = 0.5 - 0.5*cos(2*pi*(j+2)/1024),  alpha = [1, 2, ..., 2, 1] / 16382
  The fft_convolve + crop is then an ordinary linear convolution of noise row 0
  with each batch row's impulse:
      out[b, t] = amps_b / M * sum_i f[b, i] * n0u[t - 512 + i]
      f[b, i] = impulse[b, 1022 - i] (reversal folded into C), n0u = mean_0 + eps[0]
      M = max(mean + eps)  (global max over the full batch)

Distribution over 8 cores: data-parallel over batch (128 rows/core), two
phases with host-mediated exchange (no device collectives -- measured ~100us
of barrier/CC overhead for tiny payloads on this fabric):
  phase 1 (per core): MLP mean head, shard-local max of mean+eps, and a
    128-column slice of W_ic = [W_coeff; b_coeff] @ C_rev (C column-sharded,
    fp16 with a 2^13 scale, unscaled on-device after the accumulation).
  host: concatenates the 8 W_ic slices (pure gather), takes max of the 8
    shard maxima (8 floats), picks mean[0] from core 0's output.
  phase 2 (per core): amps head, filter rows fT = W_ic^T x_scaled^T with
    amps/M (and a 2^8 fp16-headroom scale) folded in, Toeplitz-matmul
    convolution against fp16 shifted windows of noise row 0, 2^-8 applied on
    the PSUM drain, streamed to the output.
"""

import os

import numpy as np

B = 1024
D = 64
DA = D + 2  # vel feats + K/MAX_STEPS + const-1 (folds the coeff bias)
W = 8192
NFFT = 16382
IRP = 1024  # impulse taps padded 1023 -> 1024
NCORES = 8
BSH = B // NCORES
MAX_STEPS = 2799.0
TPAD = 512  # left zero-padding of n0
N0PAD = 9216  # 512 + 8192 + 512 zeros
TW = 9088  # Toeplitz tile free size: 8192 + 7*128 + 512 - 512
CSCALE = 8192.0  # fp16 range lift for C (|C| <= 1.3e-4)
FSCALE = 256.0  # fp16 range lift for the conv filter rows

_CACHE = {}


def _build_crev() -> np.ndarray:
    """C_rev [8192, 1024] f32: column i equals C[:, 1022-i]; column 1023 is 0."""
    k = np.arange(W, dtype=np.float64)
    alpha = np.full(W, 2.0)
    alpha[0] = 1.0
    alpha[-1] = 1.0
    alpha /= NFFT
    j = np.arange(1023, dtype=np.float64)
    wj = 0.5 - 0.5 * np.cos(2.0 * np.pi * (j + 2.0) / 1024.0)
    ang = (2.0 * np.pi / NFFT) * np.outer(k, 7681.0 + j)
    C = (alpha[:, None] * np.cos(ang)) * wj[None, :]
    crev = np.zeros((W, IRP), dtype=np.float64)
    crev[:, :1023] = C[:, ::-1]
    return np.ascontiguousarray(crev, dtype=np.float32)


def _begin_program():
    import concourse.mybir as mybir
    import concourse.tile as tile
    from concourse import bacc

    nc = bacc.Bacc("TRN2", target_bir_lowering=False, debug=False, num_devices=NCORES)
    return nc, tile, mybir


def _build_x_aug(nc, const, vel, kk, f32):
    """x_aug [128, DA] = [vel, K/MAX_STEPS, 1]."""
    x_aug = const.tile([128, DA], f32)
    nc.sync.dma_start(x_aug[:, 0:D], vel)
    nc.sync.dma_start(x_aug[:, D : D + 1], kk)
    nc.scalar.mul(x_aug[:, D : D + 1], x_aug[:, D : D + 1], 1.0 / MAX_STEPS)
    nc.vector.memset(x_aug[:, D + 1 : DA], 1.0)
    return x_aug


def _build_prog1():
    """Per-core: mean head, local max of mean+eps, W_ic column slice."""
    from contextlib import ExitStack

    nc, tile, mybir = _begin_program()
    f32 = mybir.dt.float32
    f16 = mybir.dt.float16
    AFT = mybir.ActivationFunctionType
    X = mybir.AxisListType.X

    vel = nc.dram_tensor("vel", [BSH, D], f32, kind="ExternalInput").ap()
    kk = nc.dram_tensor("kk", [BSH, 1], f32, kind="ExternalInput").ap()
    eps_sh = nc.dram_tensor("eps_sh", [BSH, W], f32, kind="ExternalInput").ap()
    # host-prelaid SBUF layouts: wsb[p, DA*c + d] = W_aug^T[128c + p, d] (fp16)
    wsb_d = nc.dram_tensor("wsb", [128, 64 * DA], f16, kind="ExternalInput").ap()
    # csb[p, 128c + i] = CSCALE * C_rev[128c + p, my_slice + i] (fp16)
    csb_d = nc.dram_tensor("csb", [128, 64 * BSH], f16, kind="ExternalInput").ap()
    w_mean = nc.dram_tensor("w_mean", [DA, 1], f32, kind="ExternalInput").ap()
    w_amps = nc.dram_tensor("w_amps", [DA, 1], f32, kind="ExternalInput").ap()
    ident_in = nc.dram_tensor("ident", [128, 128], f32, kind="ExternalInput").ap()
    mean_out = nc.dram_tensor("mean_out", [BSH, 1], f32, kind="ExternalOutput").ap()
    amps_out = nc.dram_tensor("amps_out", [BSH, 1], f32, kind="ExternalOutput").ap()
    lmax_out = nc.dram_tensor("lmax_out", [1, 1], f32, kind="ExternalOutput").ap()
    nmean0_out = nc.dram_tensor("nmean0_out", [1, 1], f32, kind="ExternalOutput").ap()
    wic_out = nc.dram_tensor("wic_out", [DA, BSH], f32, kind="ExternalOutput").ap()

    NG = 8  # csb DMA groups for DMA/matmul overlap

    with tile.TileContext(nc) as tc, ExitStack() as ctx:
        const = ctx.enter_context(tc.tile_pool(name="const", bufs=1))
        work = ctx.enter_context(tc.tile_pool(name="work", bufs=4))
        pmisc = ctx.enter_context(tc.tile_pool(name="pmisc", bufs=2, space="PSUM"))
        pwic = ctx.enter_context(tc.tile_pool(name="pwic", bufs=1, space="PSUM"))

        # small critical loads first on the sync ring
        x_aug = _build_x_aug(nc, const, vel, kk, f32)
        wme = const.tile([DA, 1], f32)
        nc.sync.dma_start(wme[:], w_mean)
        wam = const.tile([DA, 1], f32)
        nc.sync.dma_start(wam[:], w_amps)

        # big streams on the scalar (ACT) HWDGE ring, ident first (tiny)
        ident = const.tile([128, 128], f32)
        nc.scalar.dma_start(ident[:], ident_in)
        wsb = const.tile([128, 64 * DA], f16)
        nc.scalar.dma_start(wsb[:], wsb_d)
        csb = const.tile([128, 64 * BSH], f16)
        GW = 64 * BSH // NG
        for g in range(NG):
            nc.scalar.dma_start(
                csb[:, GW * g : GW * (g + 1)], csb_d[:, GW * g : GW * (g + 1)]
            )

        xT_ps = pmisc.tile([DA, 128], f32, tag="misc")
        nc.tensor.transpose(xT_ps[:], x_aug[:], ident[:])
        xT = const.tile([DA, 128], f32)
        nc.vector.tensor_copy(xT[:], xT_ps[:])

        mean_ps = pmisc.tile([128, 1], f32, tag="misc")
        nc.tensor.matmul(mean_ps[:], lhsT=xT[:], rhs=wme[:], start=True, stop=True)
        mean_sb = const.tile([128, 1], f32)
        nc.scalar.activation(mean_sb[:], mean_ps[:], AFT.Tanh)
        nc.scalar.dma_start(mean_out, mean_sb[:])
        nmean_sb = const.tile([1, 1], f32)
        nc.scalar.mul(nmean_sb[:], mean_sb[0:1, 0:1], -1.0)
        nc.scalar.dma_start(nmean0_out, nmean_sb[:])

        amps_ps = pmisc.tile([128, 1], f32, tag="misc")
        nc.tensor.matmul(amps_ps[:], lhsT=xT[:], rhs=wam[:], start=True, stop=True)
        amps_sb = const.tile([128, 1], f32)
        nc.scalar.activation(amps_sb[:], amps_ps[:], AFT.Sigmoid)
        nc.scalar.dma_start(amps_out, amps_sb[:])

        # local max of (mean_b + eps_b[w]) over this shard
        rm4 = const.tile([128, 4], f32)
        for i in range(4):
            ch = work.tile([128, 2048], f32, tag="epschunk")
            nc.sync.dma_start(ch[:], eps_sh[:, 2048 * i : 2048 * (i + 1)])
            nc.vector.reduce_max(rm4[:, i : i + 1], ch[:], axis=X)
        rm1 = const.tile([128, 1], f32)
        nc.vector.reduce_max(rm1[:], rm4[:], axis=X)
        nc.vector.tensor_add(rm1[:], rm1[:], mean_sb[:])
        rmT_ps = pmisc.tile([1, 128], f32, tag="misc")
        nc.tensor.transpose(rmT_ps[:], rm1[:], ident[:])
        lmax_sb = const.tile([1, 1], f32)
        nc.vector.reduce_max(lmax_sb[:], rmT_ps[:], axis=X)
        nc.scalar.dma_start(lmax_out, lmax_sb[0:1, 0:1])

        # W_ic slice: [W_coeff; b_coeff] @ C_rev[:, my 128 cols], grouped to
        # overlap with the csb stream
        wic_ps = pwic.tile([DA, 128], f32, tag="wic")
        for c in range(64):
            nc.tensor.matmul(
                wic_ps[:],
                lhsT=wsb[:, DA * c : DA * (c + 1)],
                rhs=csb[:, BSH * c : BSH * (c + 1)],
                start=(c == 0),
                stop=(c == 63),
            )
        wic_sb = work.tile([DA, BSH], f32)
        nc.vector.tensor_scalar_mul(wic_sb[:], wic_ps[:], 1.0 / CSCALE)
        nc.scalar.dma_start(wic_out, wic_sb[:])

    nc.compile()
    return nc


def _build_prog2():
    """Per-core: scaled filter rows, Toeplitz-matmul convolution.

    The Toeplitz tile streams straight from the host-padded fp16 noise row
    (padding holds -mean0, so out-of-range taps cancel); the +mean0 term is
    folded in exactly as mean0 * F[b] (F = filter-row sums) on the PSUM drain.
    """
    from contextlib import ExitStack

    import concourse.bass as bass

    nc, tile, mybir = _begin_program()
    f32 = mybir.dt.float32
    f32r = mybir.dt.float32r
    f16 = mybir.dt.float16

    vel = nc.dram_tensor("vel", [BSH, D], f32, kind="ExternalInput").ap()
    kk = nc.dram_tensor("kk", [BSH, 1], f32, kind="ExternalInput").ap()
    eps0pad = nc.dram_tensor("eps0pad", [1, N0PAD], f16, kind="ExternalInput").ap()
    amps_in = nc.dram_tensor("amps_in", [BSH, 1], f32, kind="ExternalInput").ap()
    mean0_in = nc.dram_tensor("mean0_in", [1, 1], f32, kind="ExternalInput").ap()
    m_in = nc.dram_tensor("m_in", [1, 1], f32, kind="ExternalInput").ap()
    wic_full = nc.dram_tensor("wic_full", [DA, IRP], f32, kind="ExternalInput").ap()
    ident_in = nc.dram_tensor("ident", [128, 128], f32, kind="ExternalInput").ap()
    out_noise = nc.dram_tensor("out_noise", [BSH, W], f32, kind="ExternalOutput").ap()

    with tile.TileContext(nc) as tc, ExitStack() as ctx:
        const = ctx.enter_context(tc.tile_pool(name="const", bufs=1))
        work = ctx.enter_context(tc.tile_pool(name="work", bufs=3))
        pmisc = ctx.enter_context(tc.tile_pool(name="pmisc", bufs=2, space="PSUM"))
        pconv = ctx.enter_context(tc.tile_pool(name="pconv", bufs=4, space="PSUM"))
        pft = ctx.enter_context(tc.tile_pool(name="pft", bufs=2, space="PSUM"))

        # small loads first: DMA completion sems are FIFO per lane, so these
        # must not queue behind the big Toeplitz pieces
        m0_sb = const.tile([1, 1], f32)
        nc.sync.dma_start(m0_sb[:], mean0_in)
        amps_sb = const.tile([128, 1], f32)
        nc.sync.dma_start(amps_sb[:], amps_in)
        gmax = const.tile([1, 1], f32)
        nc.sync.dma_start(gmax[:], m_in)
        x_aug = _build_x_aug(nc, const, vel, kk, f32)
        wic_sb = const.tile([DA, IRP], f32r)
        nc.scalar.dma_start(wic_sb[:], wic_full.bitcast(f32r))
        ident = const.tile([128, 128], f32)
        nc.scalar.dma_start(ident[:], ident_in)

        # ---- Toeplitz pieces stream from the padded input ----
        t_sb = const.tile([128, TW], f16)
        NP = 4
        PW = TW // NP
        for k in range(NP):
            lo = PW * k
            hi = PW * (k + 1) if k < NP - 1 else TW
            piece = bass.AP(eps0pad.tensor, lo, [[1, 128], [1, hi - lo]])
            eng = nc.sync if k % 2 == 0 else nc.scalar
            eng.dma_start(t_sb[:, lo:hi], piece)

        ones_row = const.tile([1, 128], f32)
        nc.vector.memset(ones_row[:], 1.0)
        m0bc_ps = pmisc.tile([128, 1], f32, tag="misc")
        nc.tensor.matmul(m0bc_ps[:], lhsT=ones_row[:], rhs=m0_sb[:], start=True, stop=True)
        m0bc = const.tile([128, 1], f32)
        nc.vector.tensor_copy(m0bc[:], m0bc_ps[:])

        # s_b = FSCALE * amps_b / M  (M broadcast via ones-matmul)
        gmax_ps = pmisc.tile([128, 1], f32, tag="misc")
        nc.tensor.matmul(gmax_ps[:], lhsT=ones_row[:], rhs=gmax[:], start=True, stop=True)
        minv = const.tile([128, 1], f32)
        nc.vector.reciprocal(minv[:], gmax_ps[:])
        s_sb = const.tile([128, 1], f32)
        nc.vector.tensor_mul(s_sb[:], amps_sb[:], minv[:])
        nc.scalar.mul(s_sb[:], s_sb[:], FSCALE)

        xs = const.tile([128, DA], f32)
        nc.vector.tensor_scalar_mul(xs[:], x_aug[:], s_sb[:])
        xsT_ps = pmisc.tile([DA, 128], f32, tag="misc")
        nc.tensor.transpose(xsT_ps[:], xs[:], ident[:])
        xsT = const.tile([DA, 128], f32r)
        nc.vector.tensor_copy(xsT[:], xsT_ps[:])

        # filter rows fT[i, b] = sum_d W_ic[d, i] * xs[b, d]  (fp16, x FSCALE)
        fT = const.tile([128, IRP], f16)
        for c in range(8):
            fp = pft.tile([128, 128], f32, tag="fp")
            nc.tensor.matmul(
                fp[:],
                lhsT=wic_sb[:, 128 * c : 128 * (c + 1)],
                rhs=xsT[:],
                start=True,
                stop=True,
            )
            nc.vector.tensor_copy(fT[:, 128 * c : 128 * (c + 1)], fp[:])

        # F[b] = sum_i fT[i, b]; drain adds mean0 * F before unscaling
        ones16 = const.tile([128, 1], f16)
        nc.vector.memset(ones16[:], 1.0)
        f_ps = pmisc.tile([128, 1], f32, tag="misc")
        for c in range(8):
            nc.tensor.matmul(
                f_ps[:],
                lhsT=fT[:, 128 * c : 128 * (c + 1)],
                rhs=ones16[:],
                start=(c == 0),
                stop=(c == 7),
            )
        mF = const.tile([128, 1], f32)
        nc.vector.tensor_mul(mF[:], f_ps[:], m0bc[:])
        mFs = const.tile([128, 1], f32)
        nc.vector.tensor_scalar_mul(mFs[:], mF[:], 1.0 / FSCALE)

        # ---- conv: out[b,t] = (sum_i fT[i,b] * T[.] + FSCALE*m0*F[b]) / FSCALE
        for t in range(16):
            po = pconv.tile([128, 512], f32, tag="conv")
            for c in range(8):
                nc.tensor.matmul(
                    po[:],
                    lhsT=fT[:, 128 * c : 128 * (c + 1)],
                    rhs=t_sb[:, 128 * c + 512 * t : 128 * c + 512 * t + 512],
                    start=(c == 0),
                    stop=(c == 7),
                )
            ob = work.tile([128, 512], f32, tag="outbounce")
            if t % 2 == 0:
                nc.vector.tensor_scalar(
                    ob[:],
                    po[:],
                    mF[:],
                    1.0 / FSCALE,
                    op0=mybir.AluOpType.add,
                    op1=mybir.AluOpType.mult,
                )
            else:
                nc.scalar.activation(
                    ob[:],
                    po[:],
                    mybir.ActivationFunctionType.Identity,
                    bias=mFs[:],
                    scale=1.0 / FSCALE,
                )
            nc.scalar.dma_start(out_noise[:, 512 * t : 512 * (t + 1)], ob[:])

    nc.compile()
    return nc


def _get_progs():
    if "nc1" not in _CACHE:
        _CACHE["nc1"] = _build_prog1()
        _CACHE["nc2"] = _build_prog2()
    return _CACHE["nc1"], _CACHE["nc2"]


def _get_crev16():
    """(CSCALE * C_rev) as fp16, chunk-relaid: [8192, 1024] -> [64, 128, 1024]."""
    if "crev16" not in _CACHE:
        crev = _build_crev()
        _CACHE["crev16"] = np.ascontiguousarray(
            (crev * CSCALE).astype(np.float16).reshape(64, 128, IRP)
        )
    return _CACHE["crev16"]


def _prep(inputs: dict) -> dict:
    p = {}
    if "ident" not in _CACHE:
        _CACHE["ident"] = np.ascontiguousarray(np.eye(128, dtype=np.float32))
    p["ident"] = _CACHE["ident"]
    p["vel"] = np.ascontiguousarray(np.asarray(inputs["vel_inputs"]), dtype=np.float32)
    p["K"] = np.ascontiguousarray(np.asarray(inputs["K"]), dtype=np.float32)
    p["eps"] = np.ascontiguousarray(np.asarray(inputs["eps"]), dtype=np.float32)
    w_coeff = np.asarray(inputs["W_coeff"], dtype=np.float32)
    b_coeff = np.asarray(inputs["b_coeff"], dtype=np.float32)
    w_aug_t = np.concatenate([w_coeff.T, b_coeff[:, None]], axis=1)  # [W, DA]
    # wsb[p, DA*c + d] = w_aug_t[128c + p, d], fp16
    p["wsb"] = np.ascontiguousarray(
        w_aug_t.reshape(64, 128, DA).transpose(1, 0, 2).reshape(128, 64 * DA)
    ).astype(np.float16)
    p["w_amps"] = np.ascontiguousarray(
        np.concatenate(
            [np.asarray(inputs["W_amps"], np.float32), np.asarray(inputs["b_amps"], np.float32)[:, None]],
            axis=0,
        )
    )
    p["w_mean"] = np.ascontiguousarray(
        np.concatenate(
            [np.asarray(inputs["W_mean"], np.float32), np.asarray(inputs["b_mean"], np.float32)[:, None]],
            axis=0,
        )
    )
    return p


def make_in_maps1(p: dict) -> list:
    crev16 = _get_crev16()  # [64, 128, IRP]
    maps = []
    for c in range(NCORES):
        csb = np.ascontiguousarray(
            crev16[:, :, BSH * c : BSH * (c + 1)]
            .transpose(1, 0, 2)
            .reshape(128, 64 * BSH)
        )
        sl = slice(BSH * c, BSH * (c + 1))
        maps.append(
            {
                "vel": np.ascontiguousarray(p["vel"][sl]),
                "kk": np.ascontiguousarray(p["K"][sl]),
                "eps_sh": np.ascontiguousarray(p["eps"][sl]),
                "wsb": p["wsb"],
                "csb": csb,
                "w_mean": p["w_mean"],
                "w_amps": p["w_amps"],
                "ident": p["ident"],
            }
        )
    return maps


def glue12(results1: list) -> tuple:
    """Host-side exchange: gather W_ic slices, max of shard maxima, mean row 0."""
    wic_full = np.ascontiguousarray(
        np.concatenate([r["wic_out"] for r in results1], axis=1)
    )  # [DA, 1024]
    m = np.max([r["lmax_out"][0, 0] for r in results1]).reshape(1, 1).astype(np.float32)
    mean = np.concatenate([r["mean_out"] for r in results1], axis=0)  # [B, 1]
    mean0 = np.ascontiguousarray(mean[0:1, 0:1])
    amps = [np.ascontiguousarray(r["amps_out"]) for r in results1]
    nmean0 = results1[0]["nmean0_out"][0, 0]
    return wic_full, m, mean, mean0, amps, nmean0


def make_in_maps2(
    p: dict,
    wic_full: np.ndarray,
    m: np.ndarray,
    mean0: np.ndarray,
    amps: list,
    nmean0,
) -> list:
    eps0pad = np.full((1, N0PAD), nmean0, dtype=np.float16)
    eps0pad[0, TPAD : TPAD + W] = p["eps"][0].astype(np.float16)
    eps0pad = np.ascontiguousarray(eps0pad)
    maps = []
    for c in range(NCORES):
        sl = slice(BSH * c, BSH * (c + 1))
        maps.append(
            {
                "vel": np.ascontiguousarray(p["vel"][sl]),
                "kk": np.ascontiguousarray(p["K"][sl]),
                "eps0pad": eps0pad,
                "amps_in": amps[c],
                "mean0_in": mean0,
                "m_in": m,
                "wic_full": wic_full,
                "ident": p["ident"],
            }
        )
    return maps


def kernel(**inputs):
    from concourse.bass_utils import run_bass_kernel_spmd

    nc1, nc2 = _get_progs()
    p = _prep(inputs)
    trace = os.environ.get("NOISE_KERNEL_TRACE", "0") == "1"
    core_ids = list(range(NCORES))

    res1 = run_bass_kernel_spmd(nc1, make_in_maps1(p), core_ids=core_ids, trace=trace)
    wic_full, m, mean, mean0, amps, nmean0 = glue12(res1.results)
    res2 = run_bass_kernel_spmd(
        nc2,
        make_in_maps2(p, wic_full, m, mean0, amps, nmean0),
        core_ids=core_ids,
        trace=trace,
    )
    _CACHE["last_result1"] = res1
    _CACHE["last_result2"] = res2
    out = np.concatenate([r["out_noise"] for r in res2.results], axis=0)
    return out, mean
